# revision 43
# baseline (speedup 1.0000x reference)
"""Trainium2 Bass kernel for nn_BinaryTokenClassificationModel (segment_reduce).

Math: the reference pools token embeddings into word embeddings (mean over
contiguous runs of equal word ids), then computes
    logits[b,s,t] = src_pooled[b,s] @ w_src + tgt_pooled[b,t] @ w_tgt + b.
Because the classifier is linear, pooling and projection commute:
    src_proj[w] = sum_t A[w,t] * (tok_h[t] @ w_src)     (A = 1/count-weighted
    tgt_proj[w] = sum_t A[w,t] * (tok_h[t] @ w_tgt)      segment membership)
and the output is the outer sum src_proj[s] + tgt_proj[t] + b.

Device plan (per core = one batch row, no collectives):
  - Host casts everything to bf16 and packs one [128, ncol] blob: each
    partition row carries the 4 token chunks, the 4 precomputed membership
    tiles atw_c[t,w] = (seg[t]==w)/count, chunk 1 pre-transposed in
    128-blocks, and w_src as 6 column-blocks (wcol). Two HWDGE (sync-ring)
    DMAs with ~3-4 KB per-row descriptors stream it in; halving the bytes
    via bf16 + large descriptors gets the read side near the ~358 GB/s
    HBM-per-core limit.
  - Weight rows broadcast across partitions with K=1 ones-matmuls on the
    (otherwise idle) TensorEngine into PSUM, copied to SBUF bf16 on
    DVE/ACT. No GpSimd custom ops -> no ucode library load.
  - Chunks 2,3,0: DVE affine_mul_reduce computes u[t] = tok[t,:].w with the
    accumulator written directly as bf16. Chunk 1: u1 = six K=128,N=1
    matmuls on TensorE (tok1T blocks x wcol), overlapping the DVE chain.
  - One bf16 matmul per chunk (stationary atw_c against the u column
    broadcast along the free dim) realizes segment-pooling + outer-sum in
    the [S,T] PSUM tile; DVE adds the bias column (K=1 matmul broadcast)
    during the PSUM->SBUF copy; HWDGE stores the fp32 output.
"""

import functools

import ml_dtypes
import numpy as np

import concourse.bacc as bacc
import concourse.mybir as mybir
from concourse.bass_utils import run_bass_kernel_spmd
from concourse.tile import TileContext

# Problem geometry (hardcoded per spec)
B = 8
L_SRC = 256
L_TGT = 256
L = L_SRC + L_TGT  # 512
H = 768
P = 128            # SBUF partitions / tokens per chunk
NCHUNK = L // P    # 4
N_SRC_CHUNKS = L_SRC // P  # 2
N_CORES = 8
F32 = mybir.dt.float32
BF16 = mybir.dt.bfloat16

TOK_COLS = NCHUNK * H          # 3072


# ---------------------------------------------------------------------------
# Host-side segment bookkeeping (exact mirror of reference._pool_words)
# ---------------------------------------------------------------------------

def _segments(combined_wid, attention_mask, n_words):
    """Per-token dense run ids exactly as the reference computes them."""
    valid = (attention_mask > 0) & (combined_wid >= 0)  # [B, L]
    prev_wid = np.concatenate(
        [np.full((combined_wid.shape[0], 1), -2, dtype=combined_wid.dtype),
         combined_wid[:, :-1]], axis=1)
    prev_valid = np.concatenate(
        [np.zeros((valid.shape[0], 1), dtype=bool), valid[:, :-1]], axis=1)
    new_run = valid & ((combined_wid != prev_wid) | (~prev_valid))
    run_id = np.cumsum(new_run.astype(np.int64), axis=1) - 1  # [B, L]
    seg = np.where(valid, run_id, n_words)  # n_words = dummy slot
    return seg, valid


def _seg_weights(seg, valid, n_words):
    """1/max(count,1) weight for each token's segment (0 for invalid)."""
    Bv, Lv = seg.shape
    wgt = np.zeros((Bv, Lv), dtype=np.float32)
    for b in range(Bv):
        counts = np.bincount(seg[b][valid[b]], minlength=Lv + 1).astype(np.float32)
        inv = 1.0 / np.maximum(counts, 1.0)
        wgt[b] = np.where(valid[b] & (seg[b] < n_words), inv[np.minimum(seg[b], Lv)], 0.0)
    return wgt


# ---------------------------------------------------------------------------
# Device kernel
# ---------------------------------------------------------------------------

def _emit_body(nc, tc, S, T, aw):
    """aw = atw column width per chunk (P..block / S+T..general layout width).

    Block layout (aw=P): chunk c<2 pools into src cols, c>=2 into tgt cols.
    General layout (aw=S+T): every chunk has both src and tgt atw columns,
    and both u_src and u_tgt are computed per chunk (all via AMR).
    """
    # blob (all bf16, host-cast):
    #   block:   [tok2|tok3|wcol8 | tok0|tok1T | atw2|atw3|atw0|atw1]
    #   general: [tok2|tok3|tok0|tok1 | atw2|atw3|atw0|atw1]
    # tok1T = chunk-1 tokens transposed in 128-blocks (tok1T[p, j*128+t] =
    # tok1[t, j*128+p]); wcol[p, j] = w_src[j*128+p]. Chunk 1's projection
    # u1[t] = sum_h tok1[t,h] w_src[h] then runs as six K=128,N=1 matmuls on
    # the TensorEngine, cutting the serial DVE chain from 4 AMRs to 3.
    # AMR order 2,3,0: starts as soon as piece 1 lands; the last matmuls are
    # src-type (stationary atw preloadable).
    ORDER = (2, 3, 0, 1)
    general = aw != P
    WCOL = 0 if general else 8
    ncol = TOK_COLS + WCOL + NCHUNK * aw
    PSPLIT = 2 * H + WCOL
    blob_d = nc.declare_dram_parameter("blob", [P, ncol], BF16, isOutput=False)
    wcat_d = nc.declare_dram_parameter("wcat", [1, 2 * H + 1], BF16, isOutput=False)
    out_d = nc.declare_dram_parameter("out", [S, T], F32, isOutput=True)

    CP = mybir.ActivationFunctionType.Copy

    with (
        tc.tile_pool(name="const", bufs=1) as cpool,
        tc.tile_pool(name="blobp", bufs=1) as bpool,
        tc.tile_pool(name="prods", bufs=2) as ppool,
        tc.tile_pool(name="psum", bufs=1, space="PSUM") as pspool,
    ):
        blob_sb = bpool.tile([P, ncol], BF16)

        # weights+bias row on the ACT ring (single descriptor) so it doesn't
        # delay the token pieces on the sync ring
        wcat_bf = cpool.tile([1, 2 * H + 1], BF16)
        nc.scalar.dma_start(out=wcat_bf[:], in_=wcat_d[:])

        # token+membership load in two pieces, FIFO on the sync ring (a
        # concurrent second ring empirically slows both pieces down):
        # piece 1 = tok2|tok3|wcol; piece 2 = tok0|tok1T|atw
        nc.sync.dma_start(out=blob_sb[:, 0:PSPLIT], in_=blob_d[:, 0:PSPLIT])
        nc.sync.dma_start(out=blob_sb[:, PSPLIT:ncol], in_=blob_d[:, PSPLIT:ncol])

        ones_bf = cpool.tile([1, P], BF16)
        nc.vector.memset(ones_bf[:], 1.0)

        # broadcast w_src / w_tgt down the partitions: K=1 bf16 matmuls ->
        # PSUM (w_tgt first: AMR order is tgt chunks first; its copy rides
        # the idle DVE, w_src's rides ACT)
        wb_ps = {}
        for wi in (1, 0):
            ps = pspool.tile([P, 1024], F32, name=f"wbps_{wi}")
            for j0, j1 in ((0, 512), (512, H)):
                nc.tensor.matmul(
                    ps[:, j0:j1], ones_bf[0:1, 0:P],
                    wcat_bf[0:1, wi * H + j0:wi * H + j1],
                    start=True, stop=True)
            wb_ps[wi] = ps
        wb_sb = {}
        for wi, eng in ((1, "dve"), (0, "act")):
            wb = cpool.tile([P, H], BF16, name=f"wb_{wi}")
            if eng == "dve":
                nc.vector.tensor_copy(wb[:], wb_ps[wi][:, 0:H])
            else:
                nc.scalar.activation(wb[:], wb_ps[wi][:, 0:H], CP)
            wb_sb[wi] = wb

        # bias column b*ones[S,1] via K=1 matmul -> SBUF; added at the end
        bcol_ps = pspool.tile([S, 1], F32)
        nc.tensor.matmul(bcol_ps[:], ones_bf[0:1, 0:S],
                         wcat_bf[0:1, 2 * H:2 * H + 1], start=True, stop=True)
        bcol_sb = cpool.tile([S, 1], F32)
        nc.scalar.activation(bcol_sb[:], bcol_ps[:], CP)

        psum_out = pspool.tile([S, T], F32)
        n_mm = 2 * NCHUNK if general else NCHUNK
        ub_sb = cpool.tile([P, 2 * NCHUNK], BF16)

        # tok column offsets follow ORDER; wcol sits between tok3 and tok0
        tok_off = {2: 0, 3: H, 0: 2 * H + WCOL, 1: 3 * H + WCOL}
        atw_off = {c: TOK_COLS + WCOL + oi * aw for oi, c in enumerate(ORDER)}

        if not general:
            # chunk 1 projection on TensorE: u1_ps[t] += tok1T_j.T @ wcol_j
            wcol = blob_sb[:, 2 * H:2 * H + WCOL]
            t1 = tok_off[1]
            u1_ps = pspool.tile([P, 1], F32)
            for j in range(H // P):
                nc.tensor.matmul(
                    u1_ps[:], blob_sb[:, t1 + j * P:t1 + (j + 1) * P],
                    wcol[:, j:j + 1], start=(j == 0), stop=(j == H // P - 1))
            nc.scalar.activation(ub_sb[:, 3:4], u1_ps[:], CP)

        mm_i = 0
        for c in ORDER:
            tok_c = blob_sb[:, tok_off[c]:tok_off[c] + H]
            atw_c = blob_sb[:, atw_off[c]:atw_off[c] + aw]
            sides = ((0, True), (1, False)) if general else (
                ((0, True),) if c < N_SRC_CHUNKS else ((1, False),))
            for wi, is_src in sides:
                ubcol = ub_sb[:, mm_i:mm_i + 1]
                if general or c != 1:
                    prod = ppool.tile([P, H], BF16, name=f"prod_{mm_i}")
                    # bf16 accum write == fp32 accum + bf16 cast (the matmul
                    # consumes u in bf16 either way); skips a cast per chunk
                    with nc.allow_low_precision("u consumed in bf16 by mm"):
                        nc.vector.affine_mul_reduce(
                            out=prod[:], accum_out=ubcol, in0=tok_c,
                            in1=wb_sb[wi][:], scale=1.0, bias=0.0)
                first = mm_i == 0
                last = mm_i == n_mm - 1
                if general:
                    a = atw_c[:, 0:S] if is_src else atw_c[:, S:S + T]
                else:
                    a = atw_c
                if is_src:
                    nc.tensor.matmul(psum_out[:], a, ubcol.broadcast_to([P, T]),
                                     start=first, stop=last)
                else:
                    nc.tensor.matmul(psum_out[:], ubcol.broadcast_to([P, S]), a,
                                     start=first, stop=last)
                mm_i += 1

        out_sb = cpool.tile([S, T], F32)
        nc.vector.tensor_scalar_add(out_sb[:], psum_out[:], bcol_sb[0:S, 0:1])
        nc.sync.dma_start(out=out_d[:], in_=out_sb[:])


@functools.lru_cache(maxsize=4)
def _build(S, T, block_ok):
    nc = bacc.Bacc("TRN2", debug=False, num_devices=N_CORES)
    with TileContext(nc) as tc:
        _emit_body(nc, tc, S, T, P if block_ok else S + T)
    nc.compile()
    return nc


# ---------------------------------------------------------------------------
# Host wrapper
# ---------------------------------------------------------------------------

def _prep(inputs):
    tok_h = np.ascontiguousarray(np.asarray(inputs["tok_h"], dtype=np.float32))
    mask = np.asarray(inputs["attention_mask"])
    swid = np.asarray(inputs["source_word_ids"])
    twid = np.asarray(inputs["target_word_ids"])
    W = np.asarray(inputs["W"], dtype=np.float32)
    b = np.asarray(inputs["b"], dtype=np.float32)
    S = int(np.asarray(inputs["S"]))
    T = int(np.asarray(inputs["T"]))

    Bv, Lv, Hv = tok_h.shape
    assert (Bv, Lv, Hv) == (B, L, H), f"unexpected tok_h shape {tok_h.shape}"
    assert swid.shape == (B, L_SRC) and twid.shape == (B, L_TGT)
    assert S <= P and T <= P

    NW = S + T
    combined = np.concatenate([swid, twid], axis=1).astype(np.int64)
    seg, valid = _segments(combined, mask, NW)
    wgt = _seg_weights(seg, valid, NW)

    src_tok_seg = seg[:, :L_SRC][valid[:, :L_SRC]]
    tgt_tok_seg = seg[:, L_SRC:][valid[:, L_SRC:]]
    block_ok = bool(
        (src_tok_seg < S).all()
        and (tgt_tok_seg >= S).all() and (tgt_tok_seg < NW).all()
    )

    wcat = np.zeros((1, 2 * H + 1), dtype=np.float32)
    wcat[0, :H] = W[:H, 0]
    wcat[0, H:2 * H] = W[H:2 * H, 0]
    wcat[0, 2 * H] = b.reshape(-1)[0]
    wcat = wcat.astype(ml_dtypes.bfloat16)

    aw = P if block_ok else NW
    # atw[b, c, p, w] = wgt for the word column this token pools into
    atw = np.zeros((B, NCHUNK, P, aw), dtype=np.float32)
    bi, ti = np.nonzero(valid & (seg < NW))
    sg = seg[bi, ti]
    col = (sg - np.where(sg >= S, S, 0)) if block_ok else sg
    atw[bi, ti // P, ti % P, col] = wgt[bi, ti]

    # column layout (all bf16):
    #   block:   [tok2|tok3|wcol8 | tok0|tok1T | atw_2|atw_3|atw_0|atw_1]
    #   general: [tok2|tok3|tok0|tok1 | atw_2|atw_3|atw_0|atw_1]
    order = (2, 3, 0, 1)
    tok4 = tok_h.reshape(B, NCHUNK, P, H)
    a_cols = atw[:, order].transpose(0, 2, 1, 3).reshape(B, P, NCHUNK * aw)
    if block_ok:
        wcol8 = np.zeros((B, P, 8), dtype=np.float32)
        wcol8[:, :, :H // P] = W[:H, 0].reshape(H // P, P).T[None]
        tok1T = tok4[:, 1].transpose(0, 2, 1).reshape(
            B, H // P, P, P).transpose(0, 2, 1, 3).reshape(B, P, H)
        parts = [tok4[:, 2], tok4[:, 3], wcol8, tok4[:, 0], tok1T, a_cols]
    else:
        t_cols = tok4[:, order].transpose(0, 2, 1, 3).reshape(B, P, TOK_COLS)
        parts = [t_cols, a_cols]
    blob = np.ascontiguousarray(
        np.concatenate(parts, axis=2).astype(ml_dtypes.bfloat16))

    in_maps = []
    for i in range(N_CORES):
        bix = i % B
        in_maps.append({"blob": blob[bix], "wcat": wcat})
    return S, T, block_ok, in_maps


def kernel(**inputs):
    S, T, block_ok, in_maps = _prep(inputs)
    nc = _build(S, T, block_ok)
    res = run_bass_kernel_spmd(nc, in_maps, core_ids=list(range(N_CORES)))
    return np.stack([res.results[i]["out"] for i in range(B)], axis=0)
